# revision 1
# baseline (speedup 1.0000x reference)
"""Trainium2 Bass kernel for nn_CombinedLossI (combined Sinkhorn-KD/BCE/InfoNCE loss).

Sharding (8 NeuronCores, SPMD, data-driven roles):
  - q-shard the 6 logit tensors and `batch` (each core: [256,50,256] slices);
    b-shard the 4 embedding tensors ([32,50,256] per core).
  - Phase 1: per-core partial G_xy = X Y^T grams on PE (float32r) via
    PE-transpose tiles; row norms via ACT Square+accumulate; BCE dot
    partials / label sums via fused DVE scalar_tensor_tensor; InfoNCE
    partials. One [128,2048] AllReduce.
  - Phase 2 (uniform program; per-core `role` input selects the pair):
    3 cores run the xy Sinkhorn chain (10 damped iters + final
    extrapolation at blur^2). The xx/yy self-potential chains are
    data-independent (exp(-f_aa/rho) == 1.0f exactly) and skipped.
    BCE + final combine; tiny second AllReduce; every core writes the
    same scalar.
"""
import os
import sys
from contextlib import ExitStack

import numpy as np

if not any(os.path.isdir(os.path.join(p, "concourse")) for p in sys.path):
    for _cand in ("/opt/trn_rl_repo", os.path.expanduser("~/.axon_site/_ro/trn_rl_repo")):
        if os.path.isdir(os.path.join(_cand, "concourse")):
            sys.path.insert(0, _cand)
            break

import concourse.bass as bass
import concourse.bass_isa as bass_isa
import concourse.mybir as mybir
import concourse.tile as tile
from concourse import bacc
from concourse.bass_utils import run_bass_kernel_spmd
from concourse.masks import make_identity

F32 = mybir.dt.float32
F32R = mybir.dt.float32r
AF = mybir.ActivationFunctionType
ALU = mybir.AluOpType
AX = mybir.AxisListType

NCORES = 8
B = 256
T = 50
Q = 2048
D = 256
QS = Q // NCORES
NGRP = 10
TG = 5
SUBS = TG * QS // 128        # 10 f-subtiles per group
RHO = 500.0 ** 2
LN256 = float(np.log(256.0))
LN2 = float(np.log(2.0))

_eps_mid = [float(e) for e in
            np.exp(np.arange(2 * np.log(1.0), 2 * np.log(0.005), 2 * np.log(0.5)))]
EPS_LIST = [1.0] + _eps_mid + [0.005 ** 2]
EPS_FIN = 0.005 ** 2
W_UNB = RHO + EPS_FIN / 2.0
SUP_W, KD_W, EMB_W = 1.0, 0.01, 1.0

PAY_G = [0, 512, 1024]
PAY_X2 = 1536
PAY_Y2 = 1542
PAY_DOT = [1548, 1646, 1744]
PAY_S = 1842
PAY_V = 1940
PAY_W = 2048

LOGITS = ["logit_c", "logit_t", "logit_ensemble"]
TEACH = ["logit_teacher_c", "logit_teacher_t", "logit_teacher_ensemble"]
EMBS = ["out_h_student", "out_h_teacher", "out_d_student", "out_d_teacher"]

_NC_CACHE = {}


def _rep2(ap):
    """[1, N] AP -> [1, 2, N] with stride-0 middle dim (read-broadcast)."""
    return bass.AP(tensor=ap.tensor, offset=ap.offset,
                   ap=[ap.ap[0], [0, 2], ap.ap[-1]])


def build():
    nc = bacc.Bacc("TRN2", target_bir_lowering=False, debug=False,
                   num_devices=NCORES)

    xin = {nm: nc.declare_dram_parameter(nm, [B, T, QS], F32, isOutput=False)
           for nm in LOGITS + TEACH}
    bat_a = nc.declare_dram_parameter("batch_a", [B, T, QS], F32, isOutput=False)
    bat_b = nc.declare_dram_parameter("batch_b", [B, T, QS], F32, isOutput=False)
    emb = {nm: nc.declare_dram_parameter(nm, [B // NCORES, T, D], F32, isOutput=False)
           for nm in EMBS}
    role_in = nc.declare_dram_parameter("role", [1, 16], F32, isOutput=False)
    csel_in = nc.declare_dram_parameter("csel", [4, 512], F32, isOutput=False)
    out = nc.declare_dram_parameter("out", [1, 1], F32, isOutput=True)

    pay = nc.dram_tensor("pay", [128, PAY_W], F32)
    pay_red = nc.dram_tensor("pay_red", [128, PAY_W], F32)
    pay2 = nc.dram_tensor("pay2", [128, 4], F32)
    pay2_red = nc.dram_tensor("pay2_red", [128, 4], F32)

    with tile.TileContext(nc) as tc, ExitStack() as ctx:
        singles = ctx.enter_context(tc.tile_pool(name="singles", bufs=1))
        nat = ctx.enter_context(tc.tile_pool(name="nat", bufs=6))
        bat = ctx.enter_context(tc.tile_pool(name="bat", bufs=2))
        tsp = ctx.enter_context(tc.tile_pool(name="tsp", bufs=6))
        acc = ctx.enter_context(tc.tile_pool(name="acc", bufs=1))
        scr = ctx.enter_context(tc.tile_pool(name="scr", bufs=2))
        stage = ctx.enter_context(tc.tile_pool(name="stage", bufs=1))
        pps = ctx.enter_context(tc.tile_pool(name="pps", bufs=2, space="PSUM"))
        gps = ctx.enter_context(tc.tile_pool(name="gps", bufs=1, space="PSUM"))
        hps = ctx.enter_context(tc.tile_pool(name="hps", bufs=2, space="PSUM"))

        ident = singles.tile([128, 128], F32)
        make_identity(nc, ident)
        ones_col = singles.tile([1, 128], F32)
        nc.vector.memset(ones_col, 1.0)
        bias_ln2 = singles.tile([128, 1], F32)
        nc.vector.memset(bias_ln2, LN2)
        bias_nln256 = singles.tile([4, 1], F32)
        nc.vector.memset(bias_nln256, -LN256)
        eselt = singles.tile([4, 512], F32, tag="eselt", name="eselt")
        nc.sync.dma_start(out=eselt, in_=csel_in.ap())
        esel = [eselt[:, 128 * r:128 * (r + 1)] for r in range(4)]

        paysb = acc.tile([128, PAY_W], F32)
        nc.vector.memset(paysb, 0.0)
        s_sl = paysb[:, PAY_S:PAY_S + 98].rearrange("P (i t) -> P i t", i=2)
        v_sl = paysb[:, PAY_V:PAY_V + 98].rearrange("P (i t) -> P i t", i=2)

        x2cols = acc.tile([128, 3, 2, NGRP], F32)
        y2cols = acc.tile([128, 3, 2, NGRP], F32)

        xd = {nm: xin[nm].ap().rearrange("(sb P) t q -> sb P t q", P=128)
              for nm in LOGITS + TEACH}
        bad = bat_a.ap().rearrange("(sb P) t q -> sb P t q", P=128)
        bbd = bat_b.ap().rearrange("(sb P) t q -> sb P t q", P=128)

        gpairs = []
        for p in range(3):
            gp_t = gps.tile([128, 2, 256], F32, tag=f"gram{p}", name=f"gram{p}")
            gpairs.append(gp_t)

        # ---------------- phase 1: grams + norms + dots + labels ----------
        for g in range(NGRP):
            t0 = TG * g + 1
            t1 = min(t0 + TG, T)
            nw = t1 - t0                      # 5, last group 4
            deltas = []
            for sb in range(2):
                first = bat.tile([128, TG, QS], F32, tag="bata", name="t_bata")
                second = bat.tile([128, TG, QS], F32, tag="batb", name="t_batb")
                nc.sync.dma_start(out=first[:, :nw, :], in_=bad[sb, :, t0:t1, :])
                nc.sync.dma_start(out=second[:, :nw, :], in_=bbd[sb, :, t0:t1, :])
                delta = bat.tile([128, TG, QS], F32, tag="delta", name="t_delta")
                nc.vector.scalar_tensor_tensor(
                    out=delta[:, :nw, :], in0=first[:, :nw, :], scalar=1.0,
                    in1=second[:, :nw, :], op0=ALU.mult, op1=ALU.add)
                fs = scr.tile([128, 2, TG], F32, tag="fs", name="t_fs")
                nc.vector.tensor_reduce(out=fs[:, 0, :nw], in_=first[:, :nw, :],
                                        axis=AX.X, op=ALU.add)
                nc.vector.tensor_reduce(out=fs[:, 1, :nw], in_=second[:, :nw, :],
                                        axis=AX.X, op=ALU.add)
                nc.vector.scalar_tensor_tensor(
                    out=s_sl[:, sb, TG * g:TG * g + nw], in0=fs[:, 0, :nw],
                    scalar=1.0, in1=fs[:, 1, :nw], op0=ALU.mult, op1=ALU.subtract)
                nc.vector.scalar_tensor_tensor(
                    out=v_sl[:, sb, TG * g:TG * g + nw], in0=fs[:, 0, :nw],
                    scalar=1.0, in1=fs[:, 1, :nw], op0=ALU.mult, op1=ALU.add)
                deltas.append(delta)
            for p in range(3):
                xts = []
                for sb in range(2):
                    xnat = nat.tile([128, TG, QS], F32, tag="xnat", name="t_xnat")
                    nc.sync.dma_start(out=xnat,
                                      in_=xd[LOGITS[p]][sb, :, TG * g:TG * (g + 1), :])
                    ynat = nat.tile([128, TG, QS], F32, tag="ynat", name="t_ynat")
                    nc.sync.dma_start(out=ynat,
                                      in_=xd[TEACH[p]][sb, :, TG * g:TG * (g + 1), :])
                    nc.scalar.activation(
                        out=scr.tile([128, TG, QS], F32, tag="sq", name="t_sq"), in_=xnat,
                        func=AF.Square, accum_out=x2cols[:, p, sb, g:g + 1])
                    nc.scalar.activation(
                        out=scr.tile([128, TG, QS], F32, tag="sq", name="t_sq"), in_=ynat,
                        func=AF.Square, accum_out=y2cols[:, p, sb, g:g + 1])
                    xts.append((xnat, ynat))
                # transposes + gram matmuls over the 10 f-subtiles
                for sub in range(SUBS):
                    sl = slice(128 * sub, 128 * (sub + 1))
                    pt = pps.tile([128, 512], F32, tag="pt", name="t_pt")
                    nc.tensor.transpose(
                        pt[:, 0:128], xts[0][1].rearrange("P a b -> P (a b)")[:, sl], ident)
                    nc.tensor.transpose(
                        pt[:, 128:256], xts[1][1].rearrange("P a b -> P (a b)")[:, sl], ident)
                    nc.tensor.transpose(
                        pt[:, 256:384], xts[0][0].rearrange("P a b -> P (a b)")[:, sl], ident)
                    nc.tensor.transpose(
                        pt[:, 384:512], xts[1][0].rearrange("P a b -> P (a b)")[:, sl], ident)
                    tv = tsp.tile([128, 512], F32R, tag="tv", name="t_tv")
                    if sub % 3 != 2:
                        nc.vector.tensor_copy(tv, pt)
                    else:
                        nc.scalar.copy(out=tv, in_=pt)
                    fst = (g == 0 and sub == 0)
                    lst = (g == NGRP - 1 and sub == SUBS - 1)
                    nc.tensor.matmul(gpairs[p][:, 0, :], tv[:, 256:384],
                                     tv[:, 0:256], start=fst, stop=lst)
                    nc.tensor.matmul(gpairs[p][:, 1, :], tv[:, 384:512],
                                     tv[:, 0:256], start=fst, stop=lst)
                # BCE dot partials: t in [5g, 5g+nw)
                dot_sl = paysb[:, PAY_DOT[p]:PAY_DOT[p] + 98].rearrange(
                    "P (i t) -> P i t", i=2)
                for sb in range(2):
                    xnat = xts[sb][0]
                    for i in range(nw):
                        tloc = TG * g + i
                        nc.vector.scalar_tensor_tensor(
                            out=scr.tile([128, QS], F32, tag="dsc", name="t_dsc"),
                            in0=xnat[:, i, :], scalar=1.0, in1=deltas[sb][:, i, :],
                            op0=ALU.mult, op1=ALU.mult,
                            accum_out=dot_sl[:, sb, tloc:tloc + 1])

        for p in range(3):
            nc.scalar.copy(out=paysb[:, PAY_G[p]:PAY_G[p] + 512],
                           in_=gpairs[p].rearrange("P a b -> P (a b)"))

        x2f = paysb[:, PAY_X2:PAY_X2 + 6].rearrange("P (p i) -> P p i", p=3)
        y2f = paysb[:, PAY_Y2:PAY_Y2 + 6].rearrange("P (p i) -> P p i", p=3)
        for p in range(3):
            for sb in range(2):
                nc.vector.tensor_reduce(out=x2f[:, p, sb:sb + 1],
                                        in_=x2cols[:, p, sb, :], axis=AX.X, op=ALU.add)
                nc.vector.tensor_reduce(out=y2f[:, p, sb:sb + 1],
                                        in_=y2cols[:, p, sb, :], axis=AX.X, op=ALU.add)

        # ---------------- phase 1b: InfoNCE partials ----------------
        NRT = 16
        RP = 100
        estat = acc.tile([128, 7, NRT], F32)
        nc.vector.memset(estat, 0.0)
        ev = {nm: emb[nm].ap().rearrange("b t d -> (b t) d").rearrange(
            "(r P) d -> r P d", P=RP) for nm in EMBS}
        for r in range(NRT):
            tl = []
            for nm in EMBS:
                tt = nat.tile([RP, D], F32, tag="em_" + nm, name="t_em_")
                nc.sync.dma_start(out=tt, in_=ev[nm][r])
                tl.append(tt)
            u, v, n1, n2 = tl
            for di, (a_, b_) in enumerate(
                    [(u, v), (u, n1), (u, n2), (u, u), (v, v), (n1, n1), (n2, n2)]):
                nc.vector.scalar_tensor_tensor(
                    out=scr.tile([RP, D], F32, tag="esc", name="t_esc"), in0=a_, scalar=1.0,
                    in1=b_, op0=ALU.mult, op1=ALU.mult,
                    accum_out=estat[:RP, di, r:r + 1])
        # z_j = 2 * dot_j * rsqrt(ss_u*ss_j) = dot_j * exp(-0.5*ln(q) + ln2)
        zt = acc.tile([128, 3, NRT], F32)
        qt = scr.tile([128, 3, NRT], F32, tag="eq", name="t_eq")
        for j in range(3):
            nc.vector.tensor_mul(qt[:RP, j, :], estat[:RP, 3, :], estat[:RP, 4 + j, :])
        lnq = scr.tile([128, 3, NRT], F32, tag="elnq", name="t_elnq")
        nc.scalar.activation(out=lnq[:RP], in_=qt[:RP], func=AF.Ln)
        rsq = scr.tile([128, 3, NRT], F32, tag="ers", name="t_ers")
        nc.scalar.activation(out=rsq[:RP], in_=lnq[:RP], func=AF.Exp,
                             scale=-0.5, bias=bias_ln2[:RP])
        for j in range(3):
            nc.vector.tensor_mul(zt[:RP, j, :], estat[:RP, j, :], rsq[:RP, j, :])
        zmax = scr.tile([128, NRT], F32, tag="ezm", name="t_ezm")
        nc.vector.tensor_reduce(out=zmax[:RP], in_=zt[:RP].rearrange("P a b -> P b a"),
                                axis=AX.X, op=ALU.max)
        ez = scr.tile([128, 3, NRT], F32, tag="eez", name="t_eez")
        for j in range(3):
            zs_ = scr.tile([128, NRT], F32, tag="ezs", name="t_ezs")
            nc.vector.tensor_sub(zs_[:RP], zt[:RP, j, :], zmax[:RP])
            nc.scalar.activation(out=ez[:RP, j, :], in_=zs_[:RP], func=AF.Exp)
        sez = scr.tile([128, NRT], F32, tag="esez", name="t_esez")
        nc.vector.tensor_reduce(out=sez[:RP], in_=ez[:RP].rearrange("P a b -> P b a"),
                                axis=AX.X, op=ALU.add)
        lsez = scr.tile([128, NRT], F32, tag="else", name="t_else")
        nc.scalar.activation(out=lsez[:RP], in_=sez[:RP], func=AF.Ln)
        embp = acc.tile([128, 1], F32)
        nc.vector.memset(embp, 0.0)
        con = scr.tile([128, NRT], F32, tag="econ", name="t_econ")
        nc.vector.tensor_add(con[:RP], lsez[:RP], zmax[:RP])
        nc.vector.scalar_tensor_tensor(out=con[:RP], in0=con[:RP], scalar=1.0,
                                       in1=zt[:RP, 0, :], op0=ALU.mult,
                                       op1=ALU.subtract, accum_out=embp[:RP])

        # ---------------- AllReduce 1 ----------------
        nc.sync.dma_start(out=pay[:, :], in_=paysb)
        nc.gpsimd.collective_compute(
            "AllReduce", ALU.add, replica_groups=[list(range(NCORES))],
            ins=[pay[:, :]], outs=[pay_red[:, :]])
        P = acc.tile([128, PAY_W], F32)
        nc.sync.dma_start(out=P, in_=pay_red[:, :])

        rolesb = singles.tile([1, 16], F32)
        nc.sync.dma_start(out=rolesb, in_=role_in[:, :])
        roleb = singles.tile([128, 16], F32)
        nc.gpsimd.partition_broadcast(roleb, rolesb)

        # ---------------- phase 2: blend + cost matrices ----------------
        x2P = P[:, PAY_X2:PAY_X2 + 6].rearrange("P (p i) -> P p i", p=3)
        y2P = P[:, PAY_Y2:PAY_Y2 + 6].rearrange("P (p i) -> P p i", p=3)
        Gb = stage.tile([128, 2, 256], F32, tag="Gb", name="t_Gb")
        x2b = scr.tile([128, 2], F32, tag="x2b", name="t_x2b")
        y2b = scr.tile([128, 2], F32, tag="y2b", name="t_y2b")
        for p in range(3):
            r_ap = roleb[:, 1 + p:2 + p]
            gsl = P[:, PAY_G[p]:PAY_G[p] + 512].rearrange("P (a b) -> P a b", a=2)
            if p == 0:
                nc.vector.tensor_scalar(out=Gb, in0=gsl, scalar1=r_ap,
                                        scalar2=None, op0=ALU.mult)
                nc.vector.tensor_scalar(out=x2b, in0=x2P[:, 0, :], scalar1=r_ap,
                                        scalar2=None, op0=ALU.mult)
                nc.vector.tensor_scalar(out=y2b, in0=y2P[:, 0, :], scalar1=r_ap,
                                        scalar2=None, op0=ALU.mult)
            else:
                nc.vector.scalar_tensor_tensor(out=Gb, in0=gsl, scalar=r_ap,
                                               in1=Gb, op0=ALU.mult, op1=ALU.add)
                nc.vector.scalar_tensor_tensor(out=x2b, in0=x2P[:, p, :], scalar=r_ap,
                                               in1=x2b, op0=ALU.mult, op1=ALU.add)
                nc.vector.scalar_tensor_tensor(out=y2b, in0=y2P[:, p, :], scalar=r_ap,
                                               in1=y2b, op0=ALU.mult, op1=ALU.add)
        x2s = scr.tile([128, 2], F32, tag="x2s", name="t_x2s")
        nc.vector.tensor_scalar_mul(x2s, x2b, 2.0)
        y2s = scr.tile([128, 2], F32, tag="y2s", name="t_y2s")
        nc.vector.tensor_scalar_mul(y2s, y2b, 2.0)

        def rows_of(col_tile, ncols, tag):
            """[128, ncols] columns -> [ncols, 128] rows (PE transpose + evac)."""
            pt_r = pps.tile([4, 128], F32, tag="ptf", name="ptf" + tag, bufs=1)
            nc.tensor.transpose(pt_r[:ncols, :], col_tile, ident)
            rr = scr.tile([4, 128], F32, tag="rw", name="rw" + tag)
            nc.vector.tensor_copy(rr[:ncols, :], pt_r[:ncols, :])
            return rr

        def bcast_rows(hh, r0, tag):
            """H[p, ib, jh*128+jl] = hh[r0+jh, jl] via selector matmuls."""
            h = hps.tile([128, 2, 256], F32, tag="H", name="H" + tag)
            for jh in range(2):
                nc.tensor.matmul(h[:, :, 128 * jh:128 * (jh + 1)],
                                 esel[r0 + jh][:, :], _rep2(hh))
            return h

        y2rows = rows_of(y2s, 2, "y2")
        Hy2 = bcast_rows(y2rows, 0, "y2")
        CA = stage.tile([128, 2, 256], F32, tag="CA", name="t_CA")
        nc.vector.scalar_tensor_tensor(out=CA, in0=Gb, scalar=-4.0, in1=Hy2,
                                       op0=ALU.mult, op1=ALU.add)
        for ib in range(2):
            nc.scalar.activation(out=CA[:, ib, :], in_=CA[:, ib, :], func=AF.Relu,
                                 bias=x2s[:, ib:ib + 1])
        CB = stage.tile([128, 2, 256], F32, tag="CB", name="t_CB")
        for jb in range(2):
            ptc = pps.tile([128, 512], F32, tag="pt", name="t_pt")
            for a in range(2):
                nc.tensor.transpose(ptc[:, 128 * a:128 * (a + 1)],
                                    CA[:, a, 128 * jb:128 * jb + 128], ident)
            nc.vector.tensor_copy(CB[:, jb, :], ptc[:, 0:256])

        # ---------------- phase 2: sinkhorn xy chain ----------------
        fgc = acc.tile([128, 4], F32)
        nc.vector.memset(fgc, 0.0)
        fcol = fgc[:, 0:2]
        gcol = fgc[:, 2:4]

        def softmin(Cm, H, eps, tau, tag):
            M = scr.tile([128, 2, 256], F32, tag=tag + "M", name=tag + "M")
            nc.vector.scalar_tensor_tensor(out=M, in0=Cm, scalar=-1.0 / eps,
                                           in1=H, op0=ALU.mult, op1=ALU.add)
            nmax = scr.tile([128, 2], F32, tag=tag + "nm", name=tag + "nm")
            nc.vector.tensor_reduce(out=nmax, in_=M, axis=AX.X, op=ALU.max,
                                    negate=True)
            sums = scr.tile([128, 2], F32, tag=tag + "sm", name=tag + "sm")
            for ib in range(2):
                nc.scalar.activation(out=scr.tile([128, 256], F32, tag=tag + "e", name=tag + "e"),
                                     in_=M[:, ib, :], func=AF.Exp,
                                     bias=nmax[:, ib:ib + 1],
                                     accum_out=sums[:, ib:ib + 1])
            lse = scr.tile([128, 2], F32, tag=tag + "ls", name=tag + "ls")
            nc.scalar.activation(out=lse, in_=sums, func=AF.Ln)
            st = scr.tile([128, 2], F32, tag=tag + "st", name=tag + "st")
            nc.vector.tensor_sub(st, lse, nmax)
            nc.vector.tensor_scalar_mul(st, st, -eps * tau)
            return st

        for it in range(len(EPS_LIST) + 1):
            eps = EPS_LIST[it] if it < len(EPS_LIST) else EPS_FIN
            tau = 1.0 / (1.0 + eps / RHO)
            fg4 = rows_of(fgc, 4, "fg%d" % min(it, 1))
            hh = scr.tile([4, 128], F32, tag="hh", name="hh")
            nc.scalar.activation(out=hh, in_=fg4, func=AF.Identity,
                                 scale=1.0 / eps, bias=bias_nln256[:, :])
            HA = bcast_rows(hh, 2, "A%d" % min(it, 1))   # from g rows
            HB = bcast_rows(hh, 0, "B%d" % min(it, 1))   # from f rows
            ft = softmin(CA, HA, eps, tau, "A")
            gt = softmin(CB, HB, eps, tau, "Bc")
            if it < len(EPS_LIST):
                fh = scr.tile([128, 2], F32, tag="fh", name="t_fh")
                nc.vector.tensor_scalar_mul(fh, ft, 0.5)
                nc.vector.scalar_tensor_tensor(out=fcol, in0=fcol, scalar=0.5,
                                               in1=fh, op0=ALU.mult, op1=ALU.add)
                gh = scr.tile([128, 2], F32, tag="gh", name="t_gh")
                nc.vector.tensor_scalar_mul(gh, gt, 0.5)
                nc.vector.scalar_tensor_tensor(out=gcol, in0=gcol, scalar=0.5,
                                               in1=gh, op0=ALU.mult, op1=ALU.add)
            else:
                nc.vector.tensor_copy(fcol, ft)
                nc.vector.tensor_copy(gcol, gt)

        expf = scr.tile([128, 2], F32, tag="expf", name="t_expf")
        nc.scalar.activation(out=expf, in_=fcol, func=AF.Exp, scale=-1.0 / RHO)
        expg = scr.tile([128, 2], F32, tag="expg", name="t_expg")
        nc.scalar.activation(out=expg, in_=gcol, func=AF.Exp, scale=-1.0 / RHO)
        eall = scr.tile([128, 2], F32, tag="eall", name="t_eall")
        nc.vector.tensor_add(eall, expf, expg)
        esum = scr.tile([128, 1], F32, tag="esum", name="t_esum")
        nc.vector.tensor_reduce(out=esum, in_=eall, axis=AX.X, op=ALU.add)
        kdcol = scr.tile([128, 1], F32, tag="kdcol", name="t_kdcol")
        nc.vector.tensor_scalar(out=kdcol, in0=esum, scalar1=-1.0 / 256.0,
                                scalar2=4.0 / 256.0, op0=ALU.mult, op1=ALU.add)
        nc.vector.tensor_scalar(out=kdcol, in0=kdcol, scalar1=roleb[:, 0:1],
                                scalar2=None, op0=ALU.mult)

        # ---------------- phase 2: BCE (replicated) ----------------
        dsl = [P[:, PAY_DOT[p]:PAY_DOT[p] + 98] for p in range(3)]
        sP = P[:, PAY_S:PAY_S + 98]
        vP = P[:, PAY_V:PAY_V + 98]
        aa = scr.tile([128, 98], F32, tag="aa", name="t_aa")
        nc.scalar.activation(out=aa, in_=sP, func=AF.Relu)
        zsum = scr.tile([128, 98], F32, tag="zsum", name="t_zsum")
        nc.vector.tensor_add(zsum, dsl[0], dsl[1])
        nc.vector.tensor_add(zsum, zsum, dsl[2])
        spsum = scr.tile([128, 98], F32, tag="spsum", name="t_spsum")
        for p in range(3):
            ex = scr.tile([128, 98], F32, tag="bex", name="t_bex")
            nc.scalar.activation(out=ex, in_=dsl[p], func=AF.Exp)
            sp = scr.tile([128, 98], F32, tag="bsp", name="t_bsp")
            nc.scalar.activation(out=sp, in_=ex, func=AF.Ln, bias=1.0)
            if p == 0:
                nc.vector.tensor_copy(spsum, sp)
            else:
                nc.vector.tensor_add(spsum, spsum, sp)
        az = scr.tile([128, 98], F32, tag="az", name="t_az")
        nc.vector.tensor_mul(az, aa, zsum)
        term = scr.tile([128, 98], F32, tag="term", name="t_term")
        nc.vector.tensor_sub(term, spsum, az)
        nc.vector.tensor_mul(term, term, vP)
        numer = scr.tile([128, 2], F32, tag="numer", name="t_numer")
        nc.vector.tensor_reduce(out=numer,
                                in_=term.rearrange("P (i t) -> P i t", i=2),
                                axis=AX.X, op=ALU.add)
        denom = scr.tile([128, 2], F32, tag="denom", name="t_denom")
        nc.vector.tensor_reduce(out=denom,
                                in_=vP.rearrange("P (i t) -> P i t", i=2),
                                axis=AX.X, op=ALU.add)
        rden = scr.tile([128, 2], F32, tag="rden", name="t_rden")
        nc.vector.reciprocal(out=rden, in_=denom)
        pstu = scr.tile([128, 2], F32, tag="pstu", name="t_pstu")
        nc.vector.tensor_mul(pstu, numer, rden)
        supcol = scr.tile([128, 1], F32, tag="supcol", name="t_supcol")
        nc.vector.tensor_reduce(out=supcol, in_=pstu, axis=AX.X, op=ALU.add)

        # ---------------- AllReduce 2 + combine ----------------
        p2 = scr.tile([128, 4], F32, tag="p2", name="t_p2")
        nc.vector.memset(p2, 0.0)
        nc.vector.tensor_copy(p2[:, 0:1], kdcol)
        nc.vector.tensor_scalar_mul(p2[:, 1:2], supcol, 1.0 / NCORES)
        nc.vector.tensor_copy(p2[:, 2:3], embp)
        nc.sync.dma_start(out=pay2[:, :], in_=p2)
        nc.gpsimd.collective_compute(
            "AllReduce", ALU.add, replica_groups=[list(range(NCORES))],
            ins=[pay2[:, :]], outs=[pay2_red[:, :]])
        p2r = scr.tile([128, 4], F32, tag="p2r", name="t_p2r")
        nc.sync.dma_start(out=p2r, in_=pay2_red[:, :])
        tot = scr.tile([128, 1], F32, tag="tot", name="t_tot")
        nc.vector.tensor_scalar_mul(tot, p2r[:, 0:1], float(W_UNB * KD_W))
        nc.vector.scalar_tensor_tensor(out=tot, in0=p2r[:, 1:2], scalar=float(SUP_W),
                                       in1=tot, op0=ALU.mult, op1=ALU.add)
        nc.vector.scalar_tensor_tensor(out=tot, in0=p2r[:, 2:3],
                                       scalar=float(EMB_W / (B * T)),
                                       in1=tot, op0=ALU.mult, op1=ALU.add)
        totr = scr.tile([128, 1], F32, tag="totr", name="t_totr")
        nc.gpsimd.partition_all_reduce(totr, tot, channels=128,
                                       reduce_op=bass_isa.ReduceOp.add)
        osb = scr.tile([1, 1], F32, tag="osb", name="t_osb")
        nc.vector.tensor_copy(osb, totr[0:1, :])
        nc.sync.dma_start(out=out[:, :], in_=osb)

    # Force a single ACT table set: every function we use lives in
    # natural_log_exp_and_others; the default per-function set choice makes
    # the Exp<->Ln alternation reload tables ~53 times (~2.7us each).
    from concourse import bacc as _baccmod
    import concourse.hw_specs as _hw
    _orig_fn = _baccmod.get_activation_tables
    _tables = dict(_hw.get_activation_tables(nc.m.arch))
    _drop = {AF.Exp, AF.Ln, AF.Square, AF.Identity, AF.Relu, AF.Copy}
    _patched = {name: (set(fns) if name == "natural_log_exp_and_others"
                       else set(fns) - _drop)
                for name, fns in _tables.items()}
    _baccmod.get_activation_tables = lambda arch: _patched
    try:
        nc.compile()
    finally:
        _baccmod.get_activation_tables = _orig_fn
    return nc


def _shard_inputs(inputs):
    maps = []
    bs = B // NCORES
    for k in range(NCORES):
        qlo = QS * k
        m = {}
        for nm in LOGITS + TEACH:
            m[nm] = np.ascontiguousarray(inputs[nm][:, :, qlo:qlo + QS])
        m["batch_a"] = np.ascontiguousarray(inputs["batch"][:, :, qlo:qlo + QS])
        m["batch_b"] = np.ascontiguousarray(inputs["batch"][:, :, Q + qlo:Q + qlo + QS])
        for nm in EMBS:
            m[nm] = np.ascontiguousarray(inputs[nm][bs * k:bs * (k + 1)])
        csel = np.zeros((4, 512), dtype=np.float32)
        for r in range(4):
            csel[r, 128 * r:128 * (r + 1)] = 1.0
        m["csel"] = csel
        role = np.zeros((1, 16), dtype=np.float32)
        if k < 3:
            role[0, 0] = 1.0
            role[0, 1 + k] = 1.0
        m["role"] = role
        maps.append(m)
    return maps


def kernel(**inputs):
    if "nc" not in _NC_CACHE:
        _NC_CACHE["nc"] = build()
    res = run_bass_kernel_spmd(_NC_CACHE["nc"], _shard_inputs(inputs),
                               core_ids=list(range(NCORES)))
    val = np.float32(res.results[0]["out"][0, 0])
    return np.asarray(val, dtype=np.float32).reshape(())



# revision 9
# speedup vs baseline: 1.5803x; 1.5803x over previous
"""Trainium2 Bass kernel for nn_CombinedLossI (combined Sinkhorn-KD/BCE/InfoNCE loss).

v2 design (8 NeuronCores, SPMD):
  Host pre-transposes every per-core shard to put the contraction axis on
  SBUF partitions and casts to fp16 (halves HBM traffic; DVE 2x modes):
    - logits: q-shard [256,50,256] -> [(t q)=12800, b=256] f16
    - batch first/second: t in [1,50) -> [(t q)=12544, 256] f16
    - embeddings: (b t)-shard 1600 rows -> [d=256, 1600] f16
  With k on partitions, the grams G_xy = X Y^T run directly on PE (no
  PE-transposes, no PSUM->SBUF operand copies), and every sum-over-k
  (row norms x2/y2, BCE dot partials, label sums SF/SS, InfoNCE dots)
  becomes a rhs=ones matmul accumulated in PSUM (engine-time ~free).
  Squares are split across ACT / Pool(gpsimd) / DVE to balance engines.
  One fp16 [128,2048] AllReduce carries G + norms + dots + labels + embp;
  3 role cores run the damped Sinkhorn chain; a tiny second AllReduce
  sums the 3 KD scalars; every core writes the same final scalar.
"""
import os
import sys
from contextlib import ExitStack

import numpy as np

if not any(os.path.isdir(os.path.join(p, "concourse")) for p in sys.path):
    for _cand in ("/opt/trn_rl_repo", os.path.expanduser("~/.axon_site/_ro/trn_rl_repo")):
        if os.path.isdir(os.path.join(_cand, "concourse")):
            sys.path.insert(0, _cand)
            break

import concourse.bass as bass
import concourse.bass_isa as bass_isa
import concourse.mybir as mybir
import concourse.tile as tile
from concourse import bacc
from concourse.bass_utils import run_bass_kernel_spmd
from concourse.masks import make_identity

F32 = mybir.dt.float32
F16 = mybir.dt.float16
AF = mybir.ActivationFunctionType
ALU = mybir.AluOpType
AX = mybir.AxisListType

NCORES = 8
B = 256
T = 50
Q = 2048
D = 256
QS = Q // NCORES
K = T * QS             # 12800 transposed rows per logit shard
KD = (T - 1) * QS      # 12544 rows for the shifted batch tensors
SLABS = K // 128       # 100
DSLABS = KD // 128     # 98
SPG = 5                # slabs per streaming group
NGRP = SLABS // SPG    # 20
EBT = B * T // NCORES  # 1600 (b,t) rows per core for InfoNCE
NEB = 13               # ceil(1600/128) partition blocks
RHO = 500.0 ** 2
LN256 = float(np.log(256.0))
LN2 = float(np.log(2.0))
SQS = 0.125            # x2/y2 pre-scale so fp16 payload cannot overflow

_eps_mid = [float(e) for e in
            np.exp(np.arange(2 * np.log(1.0), 2 * np.log(0.005), 2 * np.log(0.5)))]
EPS_LIST = [1.0] + _eps_mid + [0.005 ** 2]
EPS_FIN = 0.005 ** 2
W_UNB = RHO + EPS_FIN / 2.0
SUP_W, KD_W, EMB_W = 1.0, 0.01, 1.0

# fp16 AllReduce payload columns
PAY_G = [0, 512, 1024]
PAY_X2 = 1536          # 3 pairs x 2 blocks (scaled by SQS)
PAY_Y2 = 1542
PAY_SF = 1548          # 2 blocks x 49
PAY_SS = 1646
PAY_DOT = 1744         # 3 pairs x 2 blocks x 49
PAY_EMB = 2038
PAY_W = 2048

LOGITS = ["logit_c", "logit_t", "logit_ensemble"]
TEACH = ["logit_teacher_c", "logit_teacher_t", "logit_teacher_ensemble"]
EMBS = ["out_h_student", "out_h_teacher", "out_d_student", "out_d_teacher"]

_NC_CACHE = {}


def _rep2(ap):
    """[1, N] AP -> [1, 2, N] with stride-0 middle dim (read-broadcast)."""
    return bass.AP(tensor=ap.tensor, offset=ap.offset,
                   ap=[ap.ap[0], [0, 2], ap.ap[-1]])


def build():
    nc = bacc.Bacc("TRN2", target_bir_lowering=False, debug=False,
                   num_devices=NCORES)

    xin = {nm: nc.declare_dram_parameter(nm, [K, QS], F16, isOutput=False)
           for nm in LOGITS + TEACH}
    bat_f = nc.declare_dram_parameter("batch_f", [KD, QS], F16, isOutput=False)
    bat_s = nc.declare_dram_parameter("batch_s", [KD, QS], F16, isOutput=False)
    emb = {nm: nc.declare_dram_parameter(nm, [D, EBT], F16, isOutput=False)
           for nm in EMBS}
    role_in = nc.declare_dram_parameter("role", [1, 16], F32, isOutput=False)
    csel_in = nc.declare_dram_parameter("csel", [4, 512], F32, isOutput=False)
    out = nc.declare_dram_parameter("out", [1, 1], F32, isOutput=True)

    pay = nc.dram_tensor("pay", [128, PAY_W], F16)
    pay_red = nc.dram_tensor("pay_red", [128, PAY_W], F16)
    pay2 = nc.dram_tensor("pay2", [128, 4], F32)
    pay2_red = nc.dram_tensor("pay2_red", [128, 4], F32)

    with tile.TileContext(nc) as tc, ExitStack() as ctx:
        singles = ctx.enter_context(tc.tile_pool(name="singles", bufs=1))
        scr = ctx.enter_context(tc.tile_pool(name="scr", bufs=2))
        acc = ctx.enter_context(tc.tile_pool(name="acc", bufs=1))

        ident = singles.tile([128, 128], F32)
        make_identity(nc, ident)
        ones1 = singles.tile([128, 1], F16)
        nc.vector.memset(ones1, 1.0)
        onessq = singles.tile([128, 1], F16)
        nc.vector.memset(onessq, SQS)
        bias_ln2 = singles.tile([128, 1], F32)
        nc.vector.memset(bias_ln2, LN2)
        bias_nln256 = singles.tile([4, 1], F32)
        nc.vector.memset(bias_nln256, -LN256)
        eselt = singles.tile([4, 512], F32, tag="eselt", name="eselt")
        nc.sync.dma_start(out=eselt, in_=csel_in.ap())
        esel = [eselt[:, 128 * r:128 * (r + 1)] for r in range(4)]
        mvalid = singles.tile([128, NEB], F32)
        nc.vector.memset(mvalid, 1.0)
        nc.vector.memset(mvalid[64:128, NEB - 1:NEB], 0.0)

        rolesb = singles.tile([1, 16], F32)
        nc.sync.dma_start(out=rolesb, in_=role_in[:, :])

        pay_sb = acc.tile([128, PAY_W], F16, tag="pay_sb", name="pay_sb")
        nc.vector.memset(pay_sb[:, PAY_EMB + 1:PAY_W], 0.0)

        xd = {nm: xin[nm].ap().rearrange("(s p) b -> p s b", p=128)
              for nm in LOGITS + TEACH}
        fd = bat_f.ap().rearrange("(s p) b -> p s b", p=128)
        sd = bat_s.ap().rearrange("(s p) b -> p s b", p=128)
        ed = {nm: emb[nm].ap().rearrange("(s p) t -> s p t", p=128)
              for nm in EMBS}

        # ---- InfoNCE embedding tiles: load once, keep resident ----
        emt = {}
        for nm in EMBS:
            tt_ = singles.tile([128, 2, EBT], F16, tag="em_" + nm, name="em_" + nm)
            nc.sync.dma_start(out=tt_[:, 0, :], in_=ed[nm][0])
            nc.sync.dma_start(out=tt_[:, 1, :], in_=ed[nm][1])
            emt[nm] = tt_

        with tc.tile_pool(name="nat", bufs=2) as nat, \
             tc.tile_pool(name="bat", bufs=2) as bat, \
             tc.tile_pool(name="dlt", bufs=2) as dlt, \
             tc.tile_pool(name="ppool", bufs=2) as ppool, \
             tc.tile_pool(name="sqp", bufs=2) as sqp, \
             tc.tile_pool(name="gps", bufs=1, space="PSUM") as gps, \
             tc.tile_pool(name="sps", bufs=1, space="PSUM") as sps, \
             tc.tile_pool(name="epr", bufs=2) as epr, \
             tc.tile_pool(name="eps_", bufs=1, space="PSUM") as eps_:

            gt = [gps.tile([128, 2, 256], F32, tag=f"gram{p}", name=f"gram{p}")
                  for p in range(3)]
            # small accumulator bank: x2(6) y2(6) SF(98) SS(98) dots(294)
            sacc = sps.tile([128, 512], F32, tag="sacc", name="sacc")
            x2c = sacc[:, 0:6].rearrange("P (p i) -> P p i", p=3)
            y2c = sacc[:, 6:12].rearrange("P (p i) -> P p i", p=3)
            sfc = sacc[:, 12:110].rearrange("P (i t) -> P i t", i=2)
            ssc = sacc[:, 110:208].rearrange("P (i t) -> P i t", i=2)
            dotc = sacc[:, 208:502].rearrange("P (p i t) -> P p i t", p=3, i=2)
            edots = eps_.tile([128, 7, NEB], F32, tag="edots", name="edots")

            # ---------------- phase 1: stream groups of 5 slabs ----------
            for g in range(NGRP):
                s0 = SPG * g
                ndv = max(0, min(SPG, DSLABS - s0))   # delta slabs this group
                ft = bat.tile([128, SPG, QS], F16, tag="bf", name="t_bf")
                st_ = bat.tile([128, SPG, QS], F16, tag="bs", name="t_bs")
                if ndv > 0:
                    nc.sync.dma_start(out=ft[:, :ndv, :], in_=fd[:, s0:s0 + ndv, :])
                    nc.sync.dma_start(out=st_[:, :ndv, :], in_=sd[:, s0:s0 + ndv, :])
                xts = []
                for p in range(3):
                    xnat = nat.tile([128, SPG, QS], F16, tag=f"x{p}", name="t_x")
                    nc.sync.dma_start(out=xnat, in_=xd[LOGITS[p]][:, s0:s0 + SPG, :])
                    ynat = nat.tile([128, SPG, QS], F16, tag=f"y{p}", name="t_y")
                    nc.sync.dma_start(out=ynat, in_=xd[TEACH[p]][:, s0:s0 + SPG, :])
                    xts.append((xnat, ynat))

                delta = None
                if ndv > 0:
                    delta = dlt.tile([128, SPG, QS], F16, tag="dl", name="t_dl")
                    nc.vector.tensor_add(delta[:, :ndv, :], ft[:, :ndv, :],
                                         st_[:, :ndv, :])
                pts = []
                for p in range(3):
                    sqx = sqp.tile([128, SPG, QS], F16, tag=f"sx{p}", name="t_sx")
                    nc.scalar.activation(out=sqx, in_=xts[p][0], func=AF.Square)
                    sqy = sqp.tile([128, SPG, QS], F16, tag=f"sy{p}", name="t_sy")
                    if p < 2:
                        nc.gpsimd.tensor_mul(sqy, xts[p][1], xts[p][1])
                    else:
                        nc.vector.tensor_mul(sqy, xts[p][1], xts[p][1])
                    pt_ = None
                    if ndv > 0:
                        pt_ = ppool.tile([128, SPG, QS], F16, tag=f"pp{p}", name="t_pp")
                        nc.vector.tensor_mul(pt_[:, :ndv, :], xts[p][0][:, :ndv, :],
                                             delta[:, :ndv, :])
                    pts.append((sqx, sqy, pt_))

                # PSUM "zero region" (2KB bank) semantics: start=True marks
                # the WHOLE bank pending-zero, so exactly one matmul per bank
                # may carry start=True (the first) and one stop=True (the
                # last); pending-zero bytes auto-overwrite on first write.
                for s in range(SPG):
                    sl = s0 + s
                    tix = sl // 2
                    for p in range(3):
                        xnat, ynat = xts[p]
                        sqx, sqy, pt_ = pts[p]
                        for blk in range(2):
                            bb = slice(128 * blk, 128 * (blk + 1))
                            nc.tensor.matmul(gt[p][:, blk, :], xnat[:, s, bb],
                                             ynat[:, s, :],
                                             start=(sl == 0 and blk == 0),
                                             stop=(sl == SLABS - 1 and blk == 1))
                            nc.tensor.matmul(x2c[:, p, blk:blk + 1], sqx[:, s, bb],
                                             onessq,
                                             start=(sl == 0 and p == 0 and blk == 0),
                                             stop=False)
                            nc.tensor.matmul(y2c[:, p, blk:blk + 1], sqy[:, s, bb],
                                             onessq, start=False,
                                             stop=(sl == SLABS - 1 and p == 2
                                                   and blk == 1))
                            if s < ndv:
                                nc.tensor.matmul(dotc[:, p, blk, tix:tix + 1],
                                                 pt_[:, s, bb], ones1,
                                                 start=False, stop=False)
                    if s < ndv:
                        for blk in range(2):
                            bb = slice(128 * blk, 128 * (blk + 1))
                            nc.tensor.matmul(sfc[:, blk, tix:tix + 1], ft[:, s, bb],
                                             ones1, start=False, stop=False)
                            nc.tensor.matmul(ssc[:, blk, tix:tix + 1], st_[:, s, bb],
                                             ones1, start=False, stop=False)

            # ---------------- phase 1b: InfoNCE dot partials ----------
            u, v, n1, n2 = [emt[nm] for nm in EMBS]
            for di, (a_, b_) in enumerate(
                    [(u, v), (u, n1), (u, n2), (u, u), (v, v), (n1, n1), (n2, n2)]):
                for sl in range(2):
                    prod = epr.tile([128, EBT], F16, tag="eprod", name="t_eprod")
                    nc.vector.tensor_mul(prod, a_[:, sl, :], b_[:, sl, :])
                    for j in range(NEB):
                        w = min(128, EBT - 128 * j)
                        nc.tensor.matmul(edots[0:w, di, j:j + 1],
                                         prod[:, 128 * j:128 * j + w], ones1,
                                         start=(di == 0 and sl == 0 and j == 0),
                                         stop=(di == 6 and sl == 1 and j == NEB - 1))

            # ---- InfoNCE tail: z/lse over [128, NEB], accum embp ----
            estat = acc.tile([128, 7, NEB], F32)
            nc.vector.tensor_copy(estat, edots)
            nc.vector.memset(estat[64:128, :, NEB - 1:NEB], 1.0)
            zt = acc.tile([128, 3, NEB], F32)
            qt = scr.tile([128, 3, NEB], F32, tag="eq", name="t_eq")
            for j in range(3):
                nc.vector.tensor_mul(qt[:, j, :], estat[:, 3, :], estat[:, 4 + j, :])
            lnq = scr.tile([128, 3, NEB], F32, tag="elnq", name="t_elnq")
            nc.scalar.activation(out=lnq, in_=qt, func=AF.Ln)
            rsq = scr.tile([128, 3, NEB], F32, tag="ers", name="t_ers")
            nc.scalar.activation(out=rsq, in_=lnq, func=AF.Exp,
                                 scale=-0.5, bias=bias_ln2)
            for j in range(3):
                nc.vector.tensor_mul(zt[:, j, :], estat[:, j, :], rsq[:, j, :])
            zmax = scr.tile([128, NEB], F32, tag="ezm", name="t_ezm")
            nc.vector.tensor_reduce(out=zmax, in_=zt.rearrange("P a b -> P b a"),
                                    axis=AX.X, op=ALU.max)
            ez = scr.tile([128, 3, NEB], F32, tag="eez", name="t_eez")
            for j in range(3):
                zs_ = scr.tile([128, NEB], F32, tag="ezs", name="t_ezs")
                nc.vector.tensor_sub(zs_, zt[:, j, :], zmax)
                nc.scalar.activation(out=ez[:, j, :], in_=zs_, func=AF.Exp)
            sez = scr.tile([128, NEB], F32, tag="esez", name="t_esez")
            nc.vector.tensor_reduce(out=sez, in_=ez.rearrange("P a b -> P b a"),
                                    axis=AX.X, op=ALU.add)
            lsez = scr.tile([128, NEB], F32, tag="else", name="t_else")
            nc.scalar.activation(out=lsez, in_=sez, func=AF.Ln)
            embp = acc.tile([128, 1], F32)
            nc.vector.memset(embp, 0.0)
            con = scr.tile([128, NEB], F32, tag="econ", name="t_econ")
            nc.vector.tensor_add(con, lsez, zmax)
            nc.vector.tensor_sub(con, con, zt[:, 0, :])
            nc.vector.scalar_tensor_tensor(out=con, in0=con, scalar=1.0,
                                           in1=mvalid, op0=ALU.mult,
                                           op1=ALU.mult, accum_out=embp)

            # ---- evacuate PSUM accumulators into the fp16 payload ----
            nc.vector.tensor_copy(pay_sb[:, PAY_G[0]:PAY_G[0] + 512],
                                  gt[0].rearrange("P a b -> P (a b)"))
            nc.scalar.copy(out=pay_sb[:, PAY_G[1]:PAY_G[1] + 512],
                           in_=gt[1].rearrange("P a b -> P (a b)"))
            nc.vector.tensor_copy(pay_sb[:, PAY_G[2]:PAY_G[2] + 512],
                                  gt[2].rearrange("P a b -> P (a b)"))
            nc.vector.tensor_copy(pay_sb[:, PAY_X2:PAY_X2 + 502], sacc[:, 0:502])
            nc.vector.tensor_copy(pay_sb[:, PAY_EMB:PAY_EMB + 1], embp)

        # ---------------- AllReduce 1 (fp16) ----------------
        nc.sync.dma_start(out=pay[:, :], in_=pay_sb)
        nc.gpsimd.collective_compute(
            "AllReduce", ALU.add, replica_groups=[list(range(NCORES))],
            ins=[pay[:, :]], outs=[pay_red[:, :]])
        P = acc.tile([128, PAY_W], F16)
        nc.sync.dma_start(out=P, in_=pay_red[:, :])

        roleb = singles.tile([128, 16], F32)
        nc.gpsimd.partition_broadcast(roleb, rolesb)

        with tc.tile_pool(name="stage", bufs=1) as stage, \
             tc.tile_pool(name="pps", bufs=2, space="PSUM") as pps, \
             tc.tile_pool(name="hps", bufs=2, space="PSUM") as hps:

            # ---------------- phase 2: blend + cost matrices ----------
            x2P = P[:, PAY_X2:PAY_X2 + 6].rearrange("P (p i) -> P p i", p=3)
            y2P = P[:, PAY_Y2:PAY_Y2 + 6].rearrange("P (p i) -> P p i", p=3)
            Gb = stage.tile([128, 2, 256], F32, tag="Gb", name="t_Gb")
            x2b = scr.tile([128, 2], F32, tag="x2b", name="t_x2b")
            y2b = scr.tile([128, 2], F32, tag="y2b", name="t_y2b")
            for p in range(3):
                r_ap = roleb[:, 1 + p:2 + p]
                gsl = P[:, PAY_G[p]:PAY_G[p] + 512].rearrange("P (a b) -> P a b", a=2)
                if p == 0:
                    nc.vector.tensor_scalar(out=Gb, in0=gsl, scalar1=r_ap,
                                            scalar2=None, op0=ALU.mult)
                    nc.vector.tensor_scalar(out=x2b, in0=x2P[:, 0, :], scalar1=r_ap,
                                            scalar2=None, op0=ALU.mult)
                    nc.vector.tensor_scalar(out=y2b, in0=y2P[:, 0, :], scalar1=r_ap,
                                            scalar2=None, op0=ALU.mult)
                else:
                    nc.vector.scalar_tensor_tensor(out=Gb, in0=gsl, scalar=r_ap,
                                                   in1=Gb, op0=ALU.mult, op1=ALU.add)
                    nc.vector.scalar_tensor_tensor(out=x2b, in0=x2P[:, p, :],
                                                   scalar=r_ap, in1=x2b,
                                                   op0=ALU.mult, op1=ALU.add)
                    nc.vector.scalar_tensor_tensor(out=y2b, in0=y2P[:, p, :],
                                                   scalar=r_ap, in1=y2b,
                                                   op0=ALU.mult, op1=ALU.add)
            x2s = scr.tile([128, 2], F32, tag="x2s", name="t_x2s")
            nc.vector.tensor_scalar_mul(x2s, x2b, 2.0 / SQS)
            y2s = scr.tile([128, 2], F32, tag="y2s", name="t_y2s")
            nc.vector.tensor_scalar_mul(y2s, y2b, 2.0 / SQS)

            def rows_of(col_tile, ncols, tag):
                """[128, ncols] columns -> [ncols, 128] rows (PE transpose)."""
                pt_r = pps.tile([4, 128], F32, tag="ptf", name="ptf" + tag, bufs=1)
                nc.tensor.transpose(pt_r[:ncols, :], col_tile, ident)
                rr = scr.tile([4, 128], F32, tag="rw", name="rw" + tag)
                nc.vector.tensor_copy(rr[:ncols, :], pt_r[:ncols, :])
                return rr

            def bcast_rows(hh, r0, tag):
                """H[p, ib, jh*128+jl] = hh[r0+jh, jl] via selector matmuls."""
                h = hps.tile([128, 2, 256], F32, tag="H", name="H" + tag)
                for jh in range(2):
                    nc.tensor.matmul(h[:, :, 128 * jh:128 * (jh + 1)],
                                     esel[r0 + jh][:, :], _rep2(hh))
                return h

            y2rows = rows_of(y2s, 2, "y2")
            Hy2 = bcast_rows(y2rows, 0, "y2")
            CA = stage.tile([128, 2, 256], F32, tag="CA", name="t_CA")
            nc.vector.scalar_tensor_tensor(out=CA, in0=Gb, scalar=-4.0, in1=Hy2,
                                           op0=ALU.mult, op1=ALU.add)
            for ib in range(2):
                nc.scalar.activation(out=CA[:, ib, :], in_=CA[:, ib, :], func=AF.Relu,
                                     bias=x2s[:, ib:ib + 1])
            CB = stage.tile([128, 2, 256], F32, tag="CB", name="t_CB")
            for jb in range(2):
                ptc = pps.tile([128, 512], F32, tag="pt", name="t_pt")
                for a in range(2):
                    nc.tensor.transpose(ptc[:, 128 * a:128 * (a + 1)],
                                        CA[:, a, 128 * jb:128 * jb + 128], ident)
                nc.vector.tensor_copy(CB[:, jb, :], ptc[:, 0:256])

            # ---------------- phase 2: sinkhorn xy chain ----------------
            fgc = acc.tile([128, 4], F32)
            nc.vector.memset(fgc, 0.0)
            fcol = fgc[:, 0:2]
            gcol = fgc[:, 2:4]

            def softmin(Cm, H, eps, tau, tag):
                M = scr.tile([128, 2, 256], F32, tag=tag + "M", name=tag + "M")
                nc.vector.scalar_tensor_tensor(out=M, in0=Cm, scalar=-1.0 / eps,
                                               in1=H, op0=ALU.mult, op1=ALU.add)
                nmax = scr.tile([128, 2], F32, tag=tag + "nm", name=tag + "nm")
                nc.vector.tensor_reduce(out=nmax, in_=M, axis=AX.X, op=ALU.max,
                                        negate=True)
                sums = scr.tile([128, 2], F32, tag=tag + "sm", name=tag + "sm")
                for ib in range(2):
                    nc.scalar.activation(
                        out=scr.tile([128, 256], F32, tag=tag + "e", name=tag + "e"),
                        in_=M[:, ib, :], func=AF.Exp, bias=nmax[:, ib:ib + 1],
                        accum_out=sums[:, ib:ib + 1])
                lse = scr.tile([128, 2], F32, tag=tag + "ls", name=tag + "ls")
                nc.scalar.activation(out=lse, in_=sums, func=AF.Ln)
                st = scr.tile([128, 2], F32, tag=tag + "st", name=tag + "st")
                nc.vector.tensor_sub(st, lse, nmax)
                nc.vector.tensor_scalar_mul(st, st, -eps * tau)
                return st

            for it in range(len(EPS_LIST) + 1):
                eps = EPS_LIST[it] if it < len(EPS_LIST) else EPS_FIN
                tau = 1.0 / (1.0 + eps / RHO)
                fg4 = rows_of(fgc, 4, "fg%d" % min(it, 1))
                hh = scr.tile([4, 128], F32, tag="hh", name="hh")
                nc.scalar.activation(out=hh, in_=fg4, func=AF.Identity,
                                     scale=1.0 / eps, bias=bias_nln256[:, :])
                HA = bcast_rows(hh, 2, "A%d" % min(it, 1))   # from g rows
                HB = bcast_rows(hh, 0, "B%d" % min(it, 1))   # from f rows
                ft = softmin(CA, HA, eps, tau, "A")
                gt_ = softmin(CB, HB, eps, tau, "Bc")
                if it < len(EPS_LIST):
                    fh = scr.tile([128, 2], F32, tag="fh", name="t_fh")
                    nc.vector.tensor_scalar_mul(fh, ft, 0.5)
                    nc.vector.scalar_tensor_tensor(out=fcol, in0=fcol, scalar=0.5,
                                                   in1=fh, op0=ALU.mult, op1=ALU.add)
                    gh = scr.tile([128, 2], F32, tag="gh", name="t_gh")
                    nc.vector.tensor_scalar_mul(gh, gt_, 0.5)
                    nc.vector.scalar_tensor_tensor(out=gcol, in0=gcol, scalar=0.5,
                                                   in1=gh, op0=ALU.mult, op1=ALU.add)
                else:
                    nc.vector.tensor_copy(fcol, ft)
                    nc.vector.tensor_copy(gcol, gt_)

            expf = scr.tile([128, 2], F32, tag="expf", name="t_expf")
            nc.scalar.activation(out=expf, in_=fcol, func=AF.Exp, scale=-1.0 / RHO)
            expg = scr.tile([128, 2], F32, tag="expg", name="t_expg")
            nc.scalar.activation(out=expg, in_=gcol, func=AF.Exp, scale=-1.0 / RHO)
            eall = scr.tile([128, 2], F32, tag="eall", name="t_eall")
            nc.vector.tensor_add(eall, expf, expg)
            esum = scr.tile([128, 1], F32, tag="esum", name="t_esum")
            nc.vector.tensor_reduce(out=esum, in_=eall, axis=AX.X, op=ALU.add)
            kdcol = scr.tile([128, 1], F32, tag="kdcol", name="t_kdcol")
            nc.vector.tensor_scalar(out=kdcol, in0=esum, scalar1=-1.0 / 256.0,
                                    scalar2=4.0 / 256.0, op0=ALU.mult, op1=ALU.add)
            nc.vector.tensor_scalar(out=kdcol, in0=kdcol, scalar1=roleb[:, 0:1],
                                    scalar2=None, op0=ALU.mult)

            # ---------------- phase 2: BCE (replicated) ----------------
            dsl = [P[:, PAY_DOT + 98 * p:PAY_DOT + 98 * (p + 1)] for p in range(3)]
            sfP = P[:, PAY_SF:PAY_SF + 98]
            ssP = P[:, PAY_SS:PAY_SS + 98]
            sP = scr.tile([128, 98], F32, tag="sP", name="t_sP")
            nc.vector.tensor_sub(sP, sfP, ssP)
            vP = scr.tile([128, 98], F32, tag="vP", name="t_vP")
            nc.vector.tensor_add(vP, sfP, ssP)
            aa = scr.tile([128, 98], F32, tag="aa", name="t_aa")
            nc.scalar.activation(out=aa, in_=sP, func=AF.Relu)
            zsum = scr.tile([128, 98], F32, tag="zsum", name="t_zsum")
            nc.vector.tensor_add(zsum, dsl[0], dsl[1])
            nc.vector.tensor_add(zsum, zsum, dsl[2])
            spsum = scr.tile([128, 98], F32, tag="spsum", name="t_spsum")
            for p in range(3):
                ex = scr.tile([128, 98], F32, tag="bex", name="t_bex")
                nc.scalar.activation(out=ex, in_=dsl[p], func=AF.Exp)
                sp = scr.tile([128, 98], F32, tag="bsp", name="t_bsp")
                nc.scalar.activation(out=sp, in_=ex, func=AF.Ln, bias=1.0)
                if p == 0:
                    nc.vector.tensor_copy(spsum, sp)
                else:
                    nc.vector.tensor_add(spsum, spsum, sp)
            az = scr.tile([128, 98], F32, tag="az", name="t_az")
            nc.vector.tensor_mul(az, aa, zsum)
            term = scr.tile([128, 98], F32, tag="term", name="t_term")
            nc.vector.tensor_sub(term, spsum, az)
            nc.vector.tensor_mul(term, term, vP)
            numer = scr.tile([128, 2], F32, tag="numer", name="t_numer")
            nc.vector.tensor_reduce(out=numer,
                                    in_=term.rearrange("P (i t) -> P i t", i=2),
                                    axis=AX.X, op=ALU.add)
            denom = scr.tile([128, 2], F32, tag="denom", name="t_denom")
            nc.vector.tensor_reduce(out=denom,
                                    in_=vP.rearrange("P (i t) -> P i t", i=2),
                                    axis=AX.X, op=ALU.add)
            rden = scr.tile([128, 2], F32, tag="rden", name="t_rden")
            nc.vector.reciprocal(out=rden, in_=denom)
            pstu = scr.tile([128, 2], F32, tag="pstu", name="t_pstu")
            nc.vector.tensor_mul(pstu, numer, rden)
            supcol = scr.tile([128, 1], F32, tag="supcol", name="t_supcol")
            nc.vector.tensor_reduce(out=supcol, in_=pstu, axis=AX.X, op=ALU.add)

            # ---------------- AllReduce 2 (kd scalars) + combine --------
            p2 = scr.tile([128, 4], F32, tag="p2", name="t_p2")
            nc.vector.memset(p2, 0.0)
            nc.vector.tensor_copy(p2[:, 0:1], kdcol)
            nc.sync.dma_start(out=pay2[:, :], in_=p2)
            nc.gpsimd.collective_compute(
                "AllReduce", ALU.add, replica_groups=[list(range(NCORES))],
                ins=[pay2[:, :]], outs=[pay2_red[:, :]])
            p2r = scr.tile([128, 4], F32, tag="p2r", name="t_p2r")
            nc.sync.dma_start(out=p2r, in_=pay2_red[:, :])
            tot = scr.tile([128, 1], F32, tag="tot", name="t_tot")
            nc.vector.tensor_scalar_mul(tot, p2r[:, 0:1], float(W_UNB * KD_W))
            nc.vector.scalar_tensor_tensor(out=tot, in0=supcol, scalar=float(SUP_W),
                                           in1=tot, op0=ALU.mult, op1=ALU.add)
            embP = scr.tile([128, 1], F32, tag="embP", name="t_embP")
            nc.vector.tensor_copy(embP, P[:, PAY_EMB:PAY_EMB + 1])
            nc.vector.scalar_tensor_tensor(out=tot, in0=embP,
                                           scalar=float(EMB_W / (B * T)),
                                           in1=tot, op0=ALU.mult, op1=ALU.add)
            totr = scr.tile([128, 1], F32, tag="totr", name="t_totr")
            nc.gpsimd.partition_all_reduce(totr, tot, channels=128,
                                           reduce_op=bass_isa.ReduceOp.add)
            osb = scr.tile([1, 1], F32, tag="osb", name="t_osb")
            nc.vector.tensor_copy(osb, totr[0:1, :])
            nc.sync.dma_start(out=out[:, :], in_=osb)

    # Force a single ACT table set (avoid Exp<->Ln table reloads).
    from concourse import bacc as _baccmod
    import concourse.hw_specs as _hw
    _orig_fn = _baccmod.get_activation_tables
    _tables = dict(_hw.get_activation_tables(nc.m.arch))
    _drop = {AF.Exp, AF.Ln, AF.Square, AF.Identity, AF.Relu, AF.Copy}
    _patched = {name: (set(fns) if name == "natural_log_exp_and_others"
                       else set(fns) - _drop)
                for name, fns in _tables.items()}
    _baccmod.get_activation_tables = lambda arch: _patched
    try:
        nc.compile()
    finally:
        _baccmod.get_activation_tables = _orig_fn
    return nc


def _shard_inputs(inputs):
    f16 = np.float16
    maps = []
    csel = np.zeros((4, 512), dtype=np.float32)
    for r in range(4):
        csel[r, 128 * r:128 * (r + 1)] = 1.0
    ebt = EBT
    for k in range(NCORES):
        qlo = QS * k
        m = {}
        for nm in LOGITS + TEACH:
            a = inputs[nm][:, :, qlo:qlo + QS].astype(f16)      # [B, T, QS]
            m[nm] = np.ascontiguousarray(a.transpose(1, 2, 0)).reshape(K, B)
        bf = inputs["batch"][:, 1:T, qlo:qlo + QS].astype(f16)
        m["batch_f"] = np.ascontiguousarray(bf.transpose(1, 2, 0)).reshape(KD, B)
        bs = inputs["batch"][:, 1:T, Q + qlo:Q + qlo + QS].astype(f16)
        m["batch_s"] = np.ascontiguousarray(bs.transpose(1, 2, 0)).reshape(KD, B)
        for nm in EMBS:
            e = inputs[nm].reshape(B * T, D)[ebt * k:ebt * (k + 1)].astype(f16)
            m[nm] = np.ascontiguousarray(e.transpose(1, 0))     # [D, EBT]
        m["csel"] = csel
        role = np.zeros((1, 16), dtype=np.float32)
        if k < 3:
            role[0, 0] = 1.0
            role[0, 1 + k] = 1.0
        m["role"] = role
        maps.append(m)
    return maps


def kernel(**inputs):
    if "nc" not in _NC_CACHE:
        _NC_CACHE["nc"] = build()
    res = run_bass_kernel_spmd(_NC_CACHE["nc"], _shard_inputs(inputs),
                               core_ids=list(range(NCORES)))
    val = np.float32(res.results[0]["out"][0, 0])
    return np.asarray(val, dtype=np.float32).reshape(())


# revision 11
# speedup vs baseline: 1.7883x; 1.1316x over previous
"""Trainium2 Bass kernel for nn_CombinedLossI (combined Sinkhorn-KD/BCE/InfoNCE loss).

v2 design (8 NeuronCores, SPMD):
  Host pre-transposes every per-core shard to put the contraction axis on
  SBUF partitions and casts to fp16 (halves HBM traffic; DVE 2x modes):
    - logits: q-shard [256,50,256] -> [(t q)=12800, b=256] f16
    - batch first/second: t in [1,50) -> [(t q)=12544, 256] f16
    - embeddings: (b t)-shard 1600 rows -> [d=256, 1600] f16
  With k on partitions, the grams G_xy = X Y^T run directly on PE (no
  PE-transposes, no PSUM->SBUF operand copies), and every sum-over-k
  (row norms x2/y2, BCE dot partials, label sums SF/SS, InfoNCE dots)
  becomes a rhs=ones matmul accumulated in PSUM (engine-time ~free).
  Squares are split across ACT / Pool(gpsimd) / DVE to balance engines.
  One fp16 [128,2048] AllReduce carries G + norms + dots + labels + embp;
  3 role cores run the damped Sinkhorn chain; a tiny second AllReduce
  sums the 3 KD scalars; every core writes the same final scalar.
"""
import os
import sys
from contextlib import ExitStack

import numpy as np

if not any(os.path.isdir(os.path.join(p, "concourse")) for p in sys.path):
    for _cand in ("/opt/trn_rl_repo", os.path.expanduser("~/.axon_site/_ro/trn_rl_repo")):
        if os.path.isdir(os.path.join(_cand, "concourse")):
            sys.path.insert(0, _cand)
            break

import concourse.bass as bass
import concourse.bass_isa as bass_isa
import concourse.mybir as mybir
import concourse.tile as tile
from concourse import bacc
from concourse.bass_utils import run_bass_kernel_spmd
from concourse.masks import make_identity

F32 = mybir.dt.float32
F16 = mybir.dt.float16
AF = mybir.ActivationFunctionType
ALU = mybir.AluOpType
AX = mybir.AxisListType

NCORES = 8
B = 256
T = 50
Q = 2048
D = 256
QS = Q // NCORES
K = T * QS             # 12800 transposed rows per logit shard
KD = (T - 1) * QS      # 12544 rows for the shifted batch tensors
SLABS = K // 128       # 100
DSLABS = KD // 128     # 98
SPG = 5                # slabs per streaming group
NGRP = SLABS // SPG    # 20
EBT = B * T // NCORES  # 1600 (b,t) rows per core for InfoNCE
NEB = 13               # ceil(1600/128) partition blocks
RHO = 500.0 ** 2
LN256 = float(np.log(256.0))
LN2 = float(np.log(2.0))
SQS = 0.125            # x2/y2 pre-scale so fp16 payload cannot overflow

_eps_mid = [float(e) for e in
            np.exp(np.arange(2 * np.log(1.0), 2 * np.log(0.005), 2 * np.log(0.5)))]
EPS_LIST = [1.0] + _eps_mid + [0.005 ** 2]
EPS_FIN = 0.005 ** 2
W_UNB = RHO + EPS_FIN / 2.0
SUP_W, KD_W, EMB_W = 1.0, 0.01, 1.0

# fp16 AllReduce payloads: pay_a (chain-critical) + pay_b (BCE-critical)
PAY_G = [0, 512, 1024]
PAY_X2 = 1536          # 3 pairs x 2 blocks (scaled by SQS)
PAY_Y2 = 1542
PAY_AW = 1548
PAY_SF = 0             # 2 blocks x 49 (pay_b)
PAY_SS = 98
PAY_DOT = 196          # 3 pairs x 2 blocks x 49
PAY_EMB = 490
PAY_BW = 512

LOGITS = ["logit_c", "logit_t", "logit_ensemble"]
TEACH = ["logit_teacher_c", "logit_teacher_t", "logit_teacher_ensemble"]
EMBS = ["out_h_student", "out_h_teacher", "out_d_student", "out_d_teacher"]

_NC_CACHE = {}


def _rep2(ap):
    """[1, N] AP -> [1, 2, N] with stride-0 middle dim (read-broadcast)."""
    return bass.AP(tensor=ap.tensor, offset=ap.offset,
                   ap=[ap.ap[0], [0, 2], ap.ap[-1]])


def build():
    nc = bacc.Bacc("TRN2", target_bir_lowering=False, debug=False,
                   num_devices=NCORES)

    xin = {nm: nc.declare_dram_parameter(nm, [K, QS], F16, isOutput=False)
           for nm in LOGITS + TEACH}
    bat_f = nc.declare_dram_parameter("batch_f", [KD, QS], F16, isOutput=False)
    bat_s = nc.declare_dram_parameter("batch_s", [KD, QS], F16, isOutput=False)
    emb = {nm: nc.declare_dram_parameter(nm, [D, EBT], F16, isOutput=False)
           for nm in EMBS}
    role_in = nc.declare_dram_parameter("role", [1, 16], F32, isOutput=False)
    csel_in = nc.declare_dram_parameter("csel", [4, 512], F32, isOutput=False)
    out = nc.declare_dram_parameter("out", [1, 1], F32, isOutput=True)

    pay_a = nc.dram_tensor("pay_a", [128, PAY_AW], F16)
    pay_a_red = nc.dram_tensor("pay_a_red", [128, PAY_AW], F16)
    pay_b = nc.dram_tensor("pay_b", [128, PAY_BW], F16)
    pay_b_red = nc.dram_tensor("pay_b_red", [128, PAY_BW], F16)
    pay2 = nc.dram_tensor("pay2", [1, 4], F32)
    pay2_red = nc.dram_tensor("pay2_red", [8, 4], F32)

    with tile.TileContext(nc) as tc, ExitStack() as ctx:
        singles = ctx.enter_context(tc.tile_pool(name="singles", bufs=1))
        scr = ctx.enter_context(tc.tile_pool(name="scr", bufs=2))
        acc = ctx.enter_context(tc.tile_pool(name="acc", bufs=1))

        ident = singles.tile([128, 128], F32)
        make_identity(nc, ident)
        ones1 = singles.tile([128, 1], F16)
        nc.vector.memset(ones1, 1.0)
        onessq = singles.tile([128, 1], F16)
        nc.vector.memset(onessq, SQS)
        bias_ln2 = singles.tile([128, 1], F32)
        nc.vector.memset(bias_ln2, LN2)
        bias_nln256 = singles.tile([4, 1], F32)
        nc.vector.memset(bias_nln256, -LN256)
        eselt = singles.tile([4, 512], F32, tag="eselt", name="eselt")
        nc.sync.dma_start(out=eselt, in_=csel_in.ap())
        esel = [eselt[:, 128 * r:128 * (r + 1)] for r in range(4)]
        mvalid = singles.tile([128, NEB], F32)
        nc.vector.memset(mvalid, 1.0)
        nc.vector.memset(mvalid[64:128, NEB - 1:NEB], 0.0)

        rolesb = singles.tile([1, 16], F32)
        nc.sync.dma_start(out=rolesb, in_=role_in[:, :])

        pay_asb = acc.tile([128, PAY_AW], F16, tag="pay_asb", name="pay_asb")
        pay_bsb = acc.tile([128, PAY_BW], F16, tag="pay_bsb", name="pay_bsb")
        nc.vector.memset(pay_bsb[:, PAY_EMB + 1:PAY_BW], 0.0)

        xd = {nm: xin[nm].ap().rearrange("(s p) b -> p s b", p=128)
              for nm in LOGITS + TEACH}
        fd = bat_f.ap().rearrange("(s p) b -> p s b", p=128)
        sd = bat_s.ap().rearrange("(s p) b -> p s b", p=128)
        ed = {nm: emb[nm].ap().rearrange("(s p) t -> s p t", p=128)
              for nm in EMBS}

        # ---- InfoNCE embedding tiles: load once, keep resident ----
        emt = {}
        for nm in EMBS:
            tt_ = singles.tile([128, 2, EBT], F16, tag="em_" + nm, name="em_" + nm)
            nc.sync.dma_start(out=tt_[:, 0, :], in_=ed[nm][0])
            nc.sync.dma_start(out=tt_[:, 1, :], in_=ed[nm][1])
            emt[nm] = tt_

        with tc.tile_pool(name="nat", bufs=2) as nat, \
             tc.tile_pool(name="bat", bufs=2) as bat, \
             tc.tile_pool(name="dlt", bufs=2) as dlt, \
             tc.tile_pool(name="ppool", bufs=2) as ppool, \
             tc.tile_pool(name="sqp", bufs=2) as sqp, \
             tc.tile_pool(name="gps", bufs=1, space="PSUM") as gps, \
             tc.tile_pool(name="sps", bufs=1, space="PSUM") as sps, \
             tc.tile_pool(name="epr", bufs=2) as epr, \
             tc.tile_pool(name="eps_", bufs=1, space="PSUM") as eps_:

            gt = [gps.tile([128, 2, 256], F32, tag=f"gram{p}", name=f"gram{p}")
                  for p in range(3)]
            # small accumulator bank: SF(98) SS(98) dots(294) x2(6) y2(6)
            sacc = sps.tile([128, 512], F32, tag="sacc", name="sacc")
            sfc = sacc[:, 0:98].rearrange("P (i t) -> P i t", i=2)
            ssc = sacc[:, 98:196].rearrange("P (i t) -> P i t", i=2)
            dotc = sacc[:, 196:490].rearrange("P (p i t) -> P p i t", p=3, i=2)
            x2c = sacc[:, 490:496].rearrange("P (p i) -> P p i", p=3)
            y2c = sacc[:, 496:502].rearrange("P (p i) -> P p i", p=3)
            edots = eps_.tile([128, 7, NEB], F32, tag="edots", name="edots")

            # ---------------- phase 1: stream groups of 5 slabs ----------
            for g in range(NGRP):
                s0 = SPG * g
                ndv = max(0, min(SPG, DSLABS - s0))   # delta slabs this group
                ft = bat.tile([128, SPG, QS], F16, tag="bf", name="t_bf")
                st_ = bat.tile([128, SPG, QS], F16, tag="bs", name="t_bs")
                if ndv > 0:
                    nc.sync.dma_start(out=ft[:, :ndv, :], in_=fd[:, s0:s0 + ndv, :])
                    nc.sync.dma_start(out=st_[:, :ndv, :], in_=sd[:, s0:s0 + ndv, :])
                xts = []
                for p in range(3):
                    xnat = nat.tile([128, SPG, QS], F16, tag=f"x{p}", name="t_x")
                    nc.sync.dma_start(out=xnat, in_=xd[LOGITS[p]][:, s0:s0 + SPG, :])
                    ynat = nat.tile([128, SPG, QS], F16, tag=f"y{p}", name="t_y")
                    nc.sync.dma_start(out=ynat, in_=xd[TEACH[p]][:, s0:s0 + SPG, :])
                    xts.append((xnat, ynat))

                delta = None
                if ndv > 0:
                    delta = dlt.tile([128, SPG, QS], F16, tag="dl", name="t_dl")
                    nc.vector.tensor_add(delta[:, :ndv, :], ft[:, :ndv, :],
                                         st_[:, :ndv, :])
                pts = []
                for p in range(3):
                    sqx = sqp.tile([128, SPG, QS], F16, tag=f"sx{p}", name="t_sx")
                    nc.scalar.activation(out=sqx, in_=xts[p][0], func=AF.Square)
                    sqy = sqp.tile([128, SPG, QS], F16, tag=f"sy{p}", name="t_sy")
                    if p < 2:
                        nc.gpsimd.tensor_mul(sqy, xts[p][1], xts[p][1])
                    else:
                        nc.vector.tensor_mul(sqy, xts[p][1], xts[p][1])
                    pt_ = None
                    if ndv > 0:
                        pt_ = ppool.tile([128, SPG, QS], F16, tag=f"pp{p}", name="t_pp")
                        nc.vector.tensor_mul(pt_[:, :ndv, :], xts[p][0][:, :ndv, :],
                                             delta[:, :ndv, :])
                    pts.append((sqx, sqy, pt_))

                # PSUM "zero region" (2KB bank) semantics: start=True marks
                # the WHOLE bank pending-zero, so exactly one matmul per bank
                # may carry start=True (the first) and one stop=True (the
                # last); pending-zero bytes auto-overwrite on first write.
                for s in range(SPG):
                    sl = s0 + s
                    tix = sl // 2
                    for p in range(3):
                        xnat, ynat = xts[p]
                        sqx, sqy, pt_ = pts[p]
                        for blk in range(2):
                            bb = slice(128 * blk, 128 * (blk + 1))
                            nc.tensor.matmul(gt[p][:, blk, :], xnat[:, s, bb],
                                             ynat[:, s, :],
                                             start=(sl == 0 and blk == 0),
                                             stop=(sl == SLABS - 1 and blk == 1))
                            nc.tensor.matmul(x2c[:, p, blk:blk + 1], sqx[:, s, bb],
                                             onessq,
                                             start=(sl == 0 and p == 0 and blk == 0),
                                             stop=False)
                            nc.tensor.matmul(y2c[:, p, blk:blk + 1], sqy[:, s, bb],
                                             onessq, start=False,
                                             stop=(sl == SLABS - 1 and p == 2
                                                   and blk == 1))
                            if s < ndv:
                                nc.tensor.matmul(dotc[:, p, blk, tix:tix + 1],
                                                 pt_[:, s, bb], ones1,
                                                 start=False, stop=False)
                    if s < ndv:
                        for blk in range(2):
                            bb = slice(128 * blk, 128 * (blk + 1))
                            nc.tensor.matmul(sfc[:, blk, tix:tix + 1], ft[:, s, bb],
                                             ones1, start=False, stop=False)
                            nc.tensor.matmul(ssc[:, blk, tix:tix + 1], st_[:, s, bb],
                                             ones1, start=False, stop=False)

            # ---------------- phase 1b: InfoNCE dot partials ----------
            u, v, n1, n2 = [emt[nm] for nm in EMBS]
            for di, (a_, b_) in enumerate(
                    [(u, v), (u, n1), (u, n2), (u, u), (v, v), (n1, n1), (n2, n2)]):
                for sl in range(2):
                    prod = epr.tile([128, EBT], F16, tag="eprod", name="t_eprod")
                    nc.vector.tensor_mul(prod, a_[:, sl, :], b_[:, sl, :])
                    for j in range(NEB):
                        w = min(128, EBT - 128 * j)
                        nc.tensor.matmul(edots[0:w, di, j:j + 1],
                                         prod[:, 128 * j:128 * j + w], ones1,
                                         start=(di == 0 and sl == 0 and j == 0),
                                         stop=(di == 6 and sl == 1 and j == NEB - 1))

            # ---- evacuate PSUM accumulators into the fp16 payloads ----
            nc.vector.tensor_copy(pay_asb[:, PAY_G[0]:PAY_G[0] + 512],
                                  gt[0].rearrange("P a b -> P (a b)"))
            nc.scalar.copy(out=pay_asb[:, PAY_G[1]:PAY_G[1] + 512],
                           in_=gt[1].rearrange("P a b -> P (a b)"))
            nc.vector.tensor_copy(pay_asb[:, PAY_G[2]:PAY_G[2] + 512],
                                  gt[2].rearrange("P a b -> P (a b)"))
            nc.vector.tensor_copy(pay_asb[:, PAY_X2:PAY_X2 + 12],
                                  sacc[:, 490:502])
            nc.sync.dma_start(out=pay_a[:, :], in_=pay_asb)
            nc.gpsimd.collective_compute(
                "AllReduce", ALU.add, replica_groups=[list(range(NCORES))],
                ins=[pay_a[:, :]], outs=[pay_a_red[:, :]])
            # ---- InfoNCE tail: z/lse over [128, NEB], accum embp ----
            estat = acc.tile([128, 7, NEB], F32)
            nc.vector.tensor_copy(estat, edots)
            nc.vector.memset(estat[64:128, :, NEB - 1:NEB], 1.0)
            zt = acc.tile([128, 3, NEB], F32)
            qt = scr.tile([128, 3, NEB], F32, tag="eq", name="t_eq")
            for j in range(3):
                nc.vector.tensor_mul(qt[:, j, :], estat[:, 3, :], estat[:, 4 + j, :])
            lnq = scr.tile([128, 3, NEB], F32, tag="elnq", name="t_elnq")
            nc.scalar.activation(out=lnq, in_=qt, func=AF.Ln)
            rsq = scr.tile([128, 3, NEB], F32, tag="ers", name="t_ers")
            nc.scalar.activation(out=rsq, in_=lnq, func=AF.Exp,
                                 scale=-0.5, bias=bias_ln2)
            for j in range(3):
                nc.vector.tensor_mul(zt[:, j, :], estat[:, j, :], rsq[:, j, :])
            zmax = scr.tile([128, NEB], F32, tag="ezm", name="t_ezm")
            nc.vector.tensor_reduce(out=zmax, in_=zt.rearrange("P a b -> P b a"),
                                    axis=AX.X, op=ALU.max)
            ez = scr.tile([128, 3, NEB], F32, tag="eez", name="t_eez")
            for j in range(3):
                zs_ = scr.tile([128, NEB], F32, tag="ezs", name="t_ezs")
                nc.vector.tensor_sub(zs_, zt[:, j, :], zmax)
                nc.scalar.activation(out=ez[:, j, :], in_=zs_, func=AF.Exp)
            sez = scr.tile([128, NEB], F32, tag="esez", name="t_esez")
            nc.vector.tensor_reduce(out=sez, in_=ez.rearrange("P a b -> P b a"),
                                    axis=AX.X, op=ALU.add)
            lsez = scr.tile([128, NEB], F32, tag="else", name="t_else")
            nc.scalar.activation(out=lsez, in_=sez, func=AF.Ln)
            embp = acc.tile([128, 1], F32)
            nc.vector.memset(embp, 0.0)
            con = scr.tile([128, NEB], F32, tag="econ", name="t_econ")
            nc.vector.tensor_add(con, lsez, zmax)
            nc.vector.tensor_sub(con, con, zt[:, 0, :])
            nc.vector.scalar_tensor_tensor(out=con, in0=con, scalar=1.0,
                                           in1=mvalid, op0=ALU.mult,
                                           op1=ALU.mult, accum_out=embp)

            nc.vector.tensor_copy(pay_bsb[:, PAY_SF:PAY_SF + 490],
                                  sacc[:, 0:490])
            nc.vector.tensor_copy(pay_bsb[:, PAY_EMB:PAY_EMB + 1], embp)

        # ---------------- AllReduce 1b + readbacks ----------------
        nc.sync.dma_start(out=pay_b[:, :], in_=pay_bsb)
        nc.gpsimd.collective_compute(
            "AllReduce", ALU.add, replica_groups=[list(range(NCORES))],
            ins=[pay_b[:, :]], outs=[pay_b_red[:, :]])
        Pa = acc.tile([128, PAY_AW], F16)
        nc.sync.dma_start(out=Pa, in_=pay_a_red[:, :])
        Pb = acc.tile([128, PAY_BW], F16)
        nc.sync.dma_start(out=Pb, in_=pay_b_red[:, :])

        roleb = singles.tile([128, 16], F32)
        nc.gpsimd.partition_broadcast(roleb, rolesb)

        with tc.tile_pool(name="stage", bufs=1) as stage, \
             tc.tile_pool(name="pps", bufs=2, space="PSUM") as pps, \
             tc.tile_pool(name="hps", bufs=2, space="PSUM") as hps:

            # ---------------- phase 2: blend + cost matrices ----------
            x2P = Pa[:, PAY_X2:PAY_X2 + 6].rearrange("P (p i) -> P p i", p=3)
            y2P = Pa[:, PAY_Y2:PAY_Y2 + 6].rearrange("P (p i) -> P p i", p=3)
            Gb = stage.tile([128, 2, 256], F32, tag="Gb", name="t_Gb")
            x2b = scr.tile([128, 2], F32, tag="x2b", name="t_x2b")
            y2b = scr.tile([128, 2], F32, tag="y2b", name="t_y2b")
            for p in range(3):
                r_ap = roleb[:, 1 + p:2 + p]
                gsl = Pa[:, PAY_G[p]:PAY_G[p] + 512].rearrange("P (a b) -> P a b", a=2)
                if p == 0:
                    nc.vector.tensor_scalar(out=Gb, in0=gsl, scalar1=r_ap,
                                            scalar2=None, op0=ALU.mult)
                    nc.vector.tensor_scalar(out=x2b, in0=x2P[:, 0, :], scalar1=r_ap,
                                            scalar2=None, op0=ALU.mult)
                    nc.vector.tensor_scalar(out=y2b, in0=y2P[:, 0, :], scalar1=r_ap,
                                            scalar2=None, op0=ALU.mult)
                else:
                    nc.vector.scalar_tensor_tensor(out=Gb, in0=gsl, scalar=r_ap,
                                                   in1=Gb, op0=ALU.mult, op1=ALU.add)
                    nc.vector.scalar_tensor_tensor(out=x2b, in0=x2P[:, p, :],
                                                   scalar=r_ap, in1=x2b,
                                                   op0=ALU.mult, op1=ALU.add)
                    nc.vector.scalar_tensor_tensor(out=y2b, in0=y2P[:, p, :],
                                                   scalar=r_ap, in1=y2b,
                                                   op0=ALU.mult, op1=ALU.add)
            x2s = scr.tile([128, 2], F32, tag="x2s", name="t_x2s")
            nc.vector.tensor_scalar_mul(x2s, x2b, 2.0 / SQS)
            y2s = scr.tile([128, 2], F32, tag="y2s", name="t_y2s")
            nc.vector.tensor_scalar_mul(y2s, y2b, 2.0 / SQS)

            def rows_of(col_tile, ncols, tag):
                """[128, ncols] columns -> [ncols, 128] rows (PE transpose)."""
                pt_r = pps.tile([4, 128], F32, tag="ptf", name="ptf" + tag, bufs=1)
                nc.tensor.transpose(pt_r[:ncols, :], col_tile, ident)
                rr = scr.tile([4, 128], F32, tag="rw", name="rw" + tag)
                nc.vector.tensor_copy(rr[:ncols, :], pt_r[:ncols, :])
                return rr

            def bcast_rows(hh, r0, tag):
                """H[p, ib, jh*128+jl] = hh[r0+jh, jl] via selector matmuls."""
                h = hps.tile([128, 2, 256], F32, tag="H", name="H" + tag)
                for jh in range(2):
                    nc.tensor.matmul(h[:, :, 128 * jh:128 * (jh + 1)],
                                     esel[r0 + jh][:, :], _rep2(hh))
                return h

            y2rows = rows_of(y2s, 2, "y2")
            Hy2 = bcast_rows(y2rows, 0, "y2")
            CA = stage.tile([128, 2, 256], F32, tag="CA", name="t_CA")
            nc.vector.scalar_tensor_tensor(out=CA, in0=Gb, scalar=-4.0, in1=Hy2,
                                           op0=ALU.mult, op1=ALU.add)
            for ib in range(2):
                nc.scalar.activation(out=CA[:, ib, :], in_=CA[:, ib, :], func=AF.Relu,
                                     bias=x2s[:, ib:ib + 1])
            CB = stage.tile([128, 2, 256], F32, tag="CB", name="t_CB")
            for jb in range(2):
                ptc = pps.tile([128, 512], F32, tag="pt", name="t_pt")
                for a in range(2):
                    nc.tensor.transpose(ptc[:, 128 * a:128 * (a + 1)],
                                        CA[:, a, 128 * jb:128 * jb + 128], ident)
                nc.vector.tensor_copy(CB[:, jb, :], ptc[:, 0:256])

            # ---------------- phase 2: sinkhorn xy chain ----------------
            fgc = acc.tile([128, 4], F32)
            nc.vector.memset(fgc, 0.0)
            fcol = fgc[:, 0:2]
            gcol = fgc[:, 2:4]

            def softmin(Cm, H, eps, tau, tag):
                # C-spread/eps >> 88 at every eps in the schedule, so the f32
                # LSE is exactly its max term: softmin = -eps*max_j(H - C/eps).
                M = scr.tile([128, 2, 256], F32, tag=tag + "M", name=tag + "M")
                nc.vector.scalar_tensor_tensor(out=M, in0=Cm, scalar=-1.0 / eps,
                                               in1=H, op0=ALU.mult, op1=ALU.add)
                nmax = scr.tile([128, 2], F32, tag=tag + "nm", name=tag + "nm")
                nc.vector.tensor_reduce(out=nmax, in_=M, axis=AX.X, op=ALU.max,
                                        negate=True)
                st = scr.tile([128, 2], F32, tag=tag + "st", name=tag + "st")
                nc.vector.tensor_scalar_mul(st, nmax, eps * tau)
                return st

            for it in range(len(EPS_LIST) + 1):
                eps = EPS_LIST[it] if it < len(EPS_LIST) else EPS_FIN
                tau = 1.0 / (1.0 + eps / RHO)
                fg4 = rows_of(fgc, 4, "fg%d" % min(it, 1))
                hh = scr.tile([4, 128], F32, tag="hh", name="hh")
                nc.scalar.activation(out=hh, in_=fg4, func=AF.Identity,
                                     scale=1.0 / eps, bias=bias_nln256[:, :])
                HA = bcast_rows(hh, 2, "A%d" % min(it, 1))   # from g rows
                HB = bcast_rows(hh, 0, "B%d" % min(it, 1))   # from f rows
                ft = softmin(CA, HA, eps, tau, "A")
                gt_ = softmin(CB, HB, eps, tau, "Bc")
                if it < len(EPS_LIST):
                    fh = scr.tile([128, 2], F32, tag="fh", name="t_fh")
                    nc.vector.tensor_scalar_mul(fh, ft, 0.5)
                    nc.vector.scalar_tensor_tensor(out=fcol, in0=fcol, scalar=0.5,
                                                   in1=fh, op0=ALU.mult, op1=ALU.add)
                    gh = scr.tile([128, 2], F32, tag="gh", name="t_gh")
                    nc.vector.tensor_scalar_mul(gh, gt_, 0.5)
                    nc.vector.scalar_tensor_tensor(out=gcol, in0=gcol, scalar=0.5,
                                                   in1=gh, op0=ALU.mult, op1=ALU.add)
                else:
                    nc.vector.tensor_copy(fcol, ft)
                    nc.vector.tensor_copy(gcol, gt_)

            expf = scr.tile([128, 2], F32, tag="expf", name="t_expf")
            nc.scalar.activation(out=expf, in_=fcol, func=AF.Exp, scale=-1.0 / RHO)
            expg = scr.tile([128, 2], F32, tag="expg", name="t_expg")
            nc.scalar.activation(out=expg, in_=gcol, func=AF.Exp, scale=-1.0 / RHO)
            eall = scr.tile([128, 2], F32, tag="eall", name="t_eall")
            nc.vector.tensor_add(eall, expf, expg)
            esum = scr.tile([128, 1], F32, tag="esum", name="t_esum")
            nc.vector.tensor_reduce(out=esum, in_=eall, axis=AX.X, op=ALU.add)
            kdcol = scr.tile([128, 1], F32, tag="kdcol", name="t_kdcol")
            nc.vector.tensor_scalar(out=kdcol, in0=esum, scalar1=-1.0 / 256.0,
                                    scalar2=4.0 / 256.0, op0=ALU.mult, op1=ALU.add)
            nc.vector.tensor_scalar(out=kdcol, in0=kdcol, scalar1=roleb[:, 0:1],
                                    scalar2=None, op0=ALU.mult)

            # ---------------- phase 2: BCE (replicated) ----------------
            dsl = [Pb[:, PAY_DOT + 98 * p:PAY_DOT + 98 * (p + 1)] for p in range(3)]
            sfP = Pb[:, PAY_SF:PAY_SF + 98]
            ssP = Pb[:, PAY_SS:PAY_SS + 98]
            sP = scr.tile([128, 98], F32, tag="sP", name="t_sP")
            nc.vector.tensor_sub(sP, sfP, ssP)
            vP = scr.tile([128, 98], F32, tag="vP", name="t_vP")
            nc.vector.tensor_add(vP, sfP, ssP)
            aa = scr.tile([128, 98], F32, tag="aa", name="t_aa")
            nc.scalar.activation(out=aa, in_=sP, func=AF.Relu)
            zsum = scr.tile([128, 98], F32, tag="zsum", name="t_zsum")
            nc.vector.tensor_add(zsum, dsl[0], dsl[1])
            nc.vector.tensor_add(zsum, zsum, dsl[2])
            spsum = scr.tile([128, 98], F32, tag="spsum", name="t_spsum")
            for p in range(3):
                ex = scr.tile([128, 98], F32, tag="bex", name="t_bex")
                nc.scalar.activation(out=ex, in_=dsl[p], func=AF.Exp)
                sp = scr.tile([128, 98], F32, tag="bsp", name="t_bsp")
                nc.scalar.activation(out=sp, in_=ex, func=AF.Ln, bias=1.0)
                if p == 0:
                    nc.vector.tensor_copy(spsum, sp)
                else:
                    nc.vector.tensor_add(spsum, spsum, sp)
            az = scr.tile([128, 98], F32, tag="az", name="t_az")
            nc.vector.tensor_mul(az, aa, zsum)
            term = scr.tile([128, 98], F32, tag="term", name="t_term")
            nc.vector.tensor_sub(term, spsum, az)
            nc.vector.tensor_mul(term, term, vP)
            numer = scr.tile([128, 2], F32, tag="numer", name="t_numer")
            nc.vector.tensor_reduce(out=numer,
                                    in_=term.rearrange("P (i t) -> P i t", i=2),
                                    axis=AX.X, op=ALU.add)
            denom = scr.tile([128, 2], F32, tag="denom", name="t_denom")
            nc.vector.tensor_reduce(out=denom,
                                    in_=vP.rearrange("P (i t) -> P i t", i=2),
                                    axis=AX.X, op=ALU.add)
            rden = scr.tile([128, 2], F32, tag="rden", name="t_rden")
            nc.vector.reciprocal(out=rden, in_=denom)
            pstu = scr.tile([128, 2], F32, tag="pstu", name="t_pstu")
            nc.vector.tensor_mul(pstu, numer, rden)
            supcol = scr.tile([128, 1], F32, tag="supcol", name="t_supcol")
            nc.vector.tensor_reduce(out=supcol, in_=pstu, axis=AX.X, op=ALU.add)

            # -------- AllGather kd scalars (cheaper than AllReduce) ------
            kdall = scr.tile([128, 1], F32, tag="kdall", name="t_kdall")
            nc.gpsimd.partition_all_reduce(kdall, kdcol, channels=128,
                                           reduce_op=bass_isa.ReduceOp.add)
            p2 = scr.tile([1, 4], F32, tag="p2", name="t_p2")
            nc.vector.memset(p2, 0.0)
            nc.vector.tensor_copy(p2[:, 0:1], kdall[0:1, :])
            nc.sync.dma_start(out=pay2[:, :], in_=p2)
            nc.gpsimd.collective_compute(
                "AllGather", ALU.bypass, replica_groups=[list(range(NCORES))],
                ins=[pay2[:, :]], outs=[pay2_red[:, :]])
            p2r = scr.tile([128, 4], F32, tag="p2r", name="t_p2r")
            nc.vector.memset(p2r, 0.0)
            nc.sync.dma_start(out=p2r[0:8, :], in_=pay2_red[:, :])
            kdg = scr.tile([128, 4], F32, tag="kdg", name="t_kdg")
            nc.gpsimd.partition_all_reduce(kdg, p2r, channels=128,
                                           reduce_op=bass_isa.ReduceOp.add)
            tot = scr.tile([128, 1], F32, tag="tot", name="t_tot")
            nc.vector.tensor_scalar(out=tot, in0=supcol, scalar1=float(SUP_W),
                                    scalar2=None, op0=ALU.mult)
            embP = scr.tile([128, 1], F32, tag="embP", name="t_embP")
            nc.vector.tensor_copy(embP, Pb[:, PAY_EMB:PAY_EMB + 1])
            nc.vector.scalar_tensor_tensor(out=tot, in0=embP,
                                           scalar=float(EMB_W / (B * T)),
                                           in1=tot, op0=ALU.mult, op1=ALU.add)
            totr = scr.tile([128, 1], F32, tag="totr", name="t_totr")
            nc.gpsimd.partition_all_reduce(totr, tot, channels=128,
                                           reduce_op=bass_isa.ReduceOp.add)
            osb = scr.tile([1, 1], F32, tag="osb", name="t_osb")
            nc.vector.scalar_tensor_tensor(out=osb, in0=kdg[0:1, 0:1],
                                           scalar=float(W_UNB * KD_W),
                                           in1=totr[0:1, :], op0=ALU.mult,
                                           op1=ALU.add)
            nc.sync.dma_start(out=out[:, :], in_=osb)

    # Force a single ACT table set (avoid Exp<->Ln table reloads).
    from concourse import bacc as _baccmod
    import concourse.hw_specs as _hw
    _orig_fn = _baccmod.get_activation_tables
    _tables = dict(_hw.get_activation_tables(nc.m.arch))
    _drop = {AF.Exp, AF.Ln, AF.Square, AF.Identity, AF.Relu, AF.Copy}
    _patched = {name: (set(fns) if name == "natural_log_exp_and_others"
                       else set(fns) - _drop)
                for name, fns in _tables.items()}
    _baccmod.get_activation_tables = lambda arch: _patched
    try:
        nc.compile()
    finally:
        _baccmod.get_activation_tables = _orig_fn
    return nc


def _shard_inputs(inputs):
    f16 = np.float16
    maps = []
    csel = np.zeros((4, 512), dtype=np.float32)
    for r in range(4):
        csel[r, 128 * r:128 * (r + 1)] = 1.0
    ebt = EBT
    for k in range(NCORES):
        qlo = QS * k
        m = {}
        for nm in LOGITS + TEACH:
            a = inputs[nm][:, :, qlo:qlo + QS].astype(f16)      # [B, T, QS]
            m[nm] = np.ascontiguousarray(a.transpose(1, 2, 0)).reshape(K, B)
        bf = inputs["batch"][:, 1:T, qlo:qlo + QS].astype(f16)
        m["batch_f"] = np.ascontiguousarray(bf.transpose(1, 2, 0)).reshape(KD, B)
        bs = inputs["batch"][:, 1:T, Q + qlo:Q + qlo + QS].astype(f16)
        m["batch_s"] = np.ascontiguousarray(bs.transpose(1, 2, 0)).reshape(KD, B)
        for nm in EMBS:
            e = inputs[nm].reshape(B * T, D)[ebt * k:ebt * (k + 1)].astype(f16)
            m[nm] = np.ascontiguousarray(e.transpose(1, 0))     # [D, EBT]
        m["csel"] = csel
        role = np.zeros((1, 16), dtype=np.float32)
        if k < 3:
            role[0, 0] = 1.0
            role[0, 1 + k] = 1.0
        m["role"] = role
        maps.append(m)
    return maps


def kernel(**inputs):
    if "nc" not in _NC_CACHE:
        _NC_CACHE["nc"] = build()
    res = run_bass_kernel_spmd(_NC_CACHE["nc"], _shard_inputs(inputs),
                               core_ids=list(range(NCORES)))
    val = np.float32(res.results[0]["out"][0, 0])
    return np.asarray(val, dtype=np.float32).reshape(())
